# revision 14
# baseline (speedup 1.0000x reference)
"""Multi-head attention (B=2, L=S=2048, D=1024, H=16, E=64) on 8 TRN2 cores.

Sharding: tensor-parallel over heads. Core c owns heads 2c, 2c+1, i.e. the
128-wide slice [c*128:(c+1)*128] of the QKV projection outputs and the
matching row-slice of Wo. Each core reads the full (host-pre-transposed)
queries/keys/values, computes its two heads' attention, and writes a full
[1024, 4096] bf16 partial of the output projection; the host sums the 8
partials, transposes back and adds bo.

v2: software-pipelined emission built around the scalar-engine exp being
the pacing engine (~133us of exp vs ~140us of matmul):
  X^T  host-supplied as [128, 8 kt, 4096 tok] bf16; DMA'd per 512-tok chunk.
  QT/KT = (XW)^T        [128 e', 512]  f32r SBUF (4 tiles per batch each)
  V'_h = [V_h | 1]      [128 s, 4*65]  bf16 (PE-transposed VT)
  scores^T (per s-tile) [128 s, 2*512 (h,l)] f32 PSUM
  P^T = exp(s/8)        [128 s, 2*512] bf16 SBUF (ScalarE)
  PV flipped: lhsT=P^T chunk [128 s, 128 l], rhs=V'_h [128 s, 65]
      -> pv[h] [128 l, 4x(65 pad to 128)] PSUM accumulated over 16 s-tiles;
      col 64 of each chunk is the softmax denominator (ones column of V').
      Free-dim is 65 instead of 512, halving PE time for PV.
  normalize: per-partition reciprocal (DVE) + tensor_scalar mult (Pool)
      -> O [128 l, 128 e'] bf16; PE-transpose -> OT [128 e', 512] bf16.
  out-proj: Wo_c.T @ OT -> [128 d, 512] PSUM -> Pool copy into ob
      [128, 8, 512] bf16 -> one batched 3D DMA per unit (DVE queue).
DMA issue is kept off the scalar queue entirely so exp dispatch never
stalls behind descriptor generation. Units are emitted as 4 quads of
(8 score matmuls + 4 exps) each, with fill work (PV waves, projections,
V-transposes, previous unit's tail) interleaved between quads in an order
matched to DMA arrival times.
"""

import numpy as np
import ml_dtypes

import concourse.bass as bass
import concourse.bacc as bacc
import concourse.mybir as mybir
from concourse.tile import TileContext
from concourse.bass_utils import run_bass_kernel_spmd

BF16 = mybir.dt.bfloat16
F32 = mybir.dt.float32
F32R = mybir.dt.float32r

B, L, D = 2, 2048, 1024
TOK = B * L              # 4096
H, E = 16, 64
NCORES = 8
E2 = 128                 # projection output dims per core (2 heads)
NKT = D // 128           # 8 k-tiles of the contraction
LQ = 512                 # l-quarter: query-token tile inside attention
NLQ = L // LQ            # 4 per batch
NST = L // 128           # 16 s-tiles (key tokens) per batch
HEADS = 2                # heads per core
NDT = D // 128           # 8 output-row tiles

_CACHED_NC = None
_IDENT = np.eye(128, dtype=ml_dtypes.bfloat16)


def _warrange(w):
    # [D, E2] -> [128, NKT*E2]: row p holds [w[kt*128+p, :] for kt]
    return np.ascontiguousarray(
        w.reshape(NKT, 128, E2).transpose(1, 0, 2).reshape(128, NKT * E2)
    ).astype(ml_dtypes.bfloat16)


def build_nc():
    nc = bacc.Bacc("TRN2", target_bir_lowering=False)

    xt = {n: nc.declare_dram_parameter(f"x{n}_t", [128, NKT, TOK], BF16,
                                       isOutput=False)
          for n in ("q", "k", "v")}
    w = {n: nc.declare_dram_parameter(f"w{n}", [128, NKT * E2], BF16,
                                      isOutput=False)
         for n in ("q", "k", "v")}
    bias = {n: nc.declare_dram_parameter(f"b{n}", [E2, 1], F32, isOutput=False)
            for n in ("q", "k", "v")}
    wo = nc.declare_dram_parameter("wo", [E2, D], BF16, isOutput=False)
    ident_in = nc.declare_dram_parameter("ident_in", [128, 128], BF16,
                                         isOutput=False)
    out_t = nc.declare_dram_parameter("out_t", [128, NDT, TOK], BF16,
                                      isOutput=True)

    units = [(0, lq) for lq in range(NLQ)] + [(1, lq) for lq in range(NLQ)]

    with TileContext(nc) as tc:
        with (
            tc.tile_pool(name="const", bufs=1) as const,
            tc.tile_pool(name="persist", bufs=1) as persist,
            tc.tile_pool(name="xt_pool", bufs=10) as xt_pool,
            tc.tile_pool(name="pt_pool", bufs=20) as pt_pool,
            tc.tile_pool(name="o_pool", bufs=8) as o_pool,
            tc.tile_pool(name="rec_pool", bufs=16) as rec_pool,
            tc.tile_pool(name="ot_pool", bufs=2) as ot_pool,
            tc.tile_pool(name="ob_pool", bufs=2) as ob_pool,
            tc.tile_pool(name="sc_ps", bufs=2, space="PSUM") as sc_ps,
            tc.tile_pool(name="pv_ps", bufs=2, space="PSUM") as pv_ps,
            tc.tile_pool(name="misc_ps", bufs=1, space="PSUM") as misc_ps,
        ):
            # ---- constants ----
            ident = const.tile([128, 128], BF16, tag="ident")
            w_sb = {n: const.tile([128, NKT * E2], BF16, tag=f"w_{n}",
                                  name=f"w_{n}") for n in ("q", "k", "v")}
            b_sb = {}
            for n in ("q", "k", "v"):
                b_dma = const.tile([E2, 1], F32, tag=f"bdma_{n}",
                                   name=f"bdma_{n}")
                nc.gpsimd.dma_start(out=b_dma[:], in_=bias[n].ap())
                b_sb[n] = const.tile([E2, 1], F32, tag=f"b_{n}", name=f"b_{n}")
                nc.vector.tensor_copy(b_sb[n][:], b_dma[:])

            warm = const.tile([1, 2], F32, tag="warm")
            nc.vector.memset(warm[:], 0.0)
            nc.scalar.activation(warm[:], warm[:],
                                 mybir.ActivationFunctionType.Exp)
            wo_sb = const.tile([E2, D], BF16, tag="wo")

            qt_sbs = [[persist.tile([E2, LQ], F32R, tag=f"qt{b}_{t}",
                                    name=f"qt{b}_{t}") for t in range(4)]
                      for b in range(B)]
            kt_sbs = [[persist.tile([E2, LQ], F32R, tag=f"kt{b}_{t}",
                                    name=f"kt{b}_{t}") for t in range(4)]
                      for b in range(B)]
            vt_sbs = [[persist.tile([E2, 512], BF16, tag=f"vt{b}_{g}",
                                    name=f"vt{b}_{g}") for g in range(4)]
                      for b in range(B)]
            # V' per head: [128 spart, (4 stile, 65)] with col 64 == 1.0
            vp_sbs = [[[persist.tile([128, 4 * 65], BF16, tag=f"vp{b}_{h}_{g}",
                                     name=f"vp{b}_{h}_{g}") for g in range(4)]
                       for h in range(HEADS)] for b in range(B)]

            for b in range(B):
                for h in range(HEADS):
                    for g in range(4):
                        nc.vector.memset(vp_sbs[b][h][g][:], 1.0)

            nc.gpsimd.dma_start(out=wo_sb[:], in_=wo.ap())
            nc.gpsimd.dma_start(out=ident[:], in_=ident_in.ap())

            proj_out = {"q": qt_sbs, "k": kt_sbs}

            # ---- DMA issue (SP queue), in arrival-need order ----
            x_sb = {}

            def dma_w(n):
                nc.gpsimd.dma_start(out=w_sb[n][:], in_=w[n].ap())

            def dma_x(n, b, tt):
                # two half-chunk DMAs into one tile: finer arrival granularity
                # at startup (DMA_ENGINES serializes transfers)
                t0 = b * L + tt * 512
                xtile = xt_pool.tile([128, NKT, 512], BF16, tag="xt",
                                     name=f"x_{n}{b}_{tt}")
                nc.sync.dma_start(out=xtile[:],
                                  in_=xt[n].ap()[:, :, t0:t0 + 512])
                x_sb[(n, b, tt)] = xtile

            dma_w("k")
            dma_w("q")
            dma_x("k", 0, 0)
            dma_x("q", 0, 0)
            dma_x("k", 0, 1)
            dma_w("v")
            dma_x("k", 0, 2)
            dma_x("k", 0, 3)
            dma_x("q", 0, 1)
            dma_x("v", 0, 0)
            dma_x("v", 0, 1)
            dma_x("v", 0, 2)
            dma_x("v", 0, 3)
            dma_x("q", 0, 2)
            dma_x("q", 0, 3)
            for key in (("k", 1, 0), ("k", 1, 1), ("k", 1, 2), ("k", 1, 3),
                        ("q", 1, 0), ("v", 1, 0), ("v", 1, 1), ("q", 1, 1),
                        ("v", 1, 2), ("v", 1, 3), ("q", 1, 2), ("q", 1, 3)):
                dma_x(*key)

            # ---- emission helpers ----
            def proj_tt(n, b, tt):
                """Project one 512-token chunk: 8 accumulating matmuls."""
                acc = misc_ps.tile([128, 512], F32, tag="op",
                                   name=f"proj_{n}{b}_{tt}")
                xtile = x_sb[(n, b, tt)]
                for kt in range(NKT):
                    nc.tensor.matmul(
                        acc[:],
                        lhsT=w_sb[n][:, kt * E2:(kt + 1) * E2],
                        rhs=xtile[:, kt, :],
                        start=(kt == 0),
                        stop=(kt == NKT - 1),
                    )
                if n == "v":
                    dst = vt_sbs[b][tt][:]
                else:
                    dst = proj_out[n][b][tt][:]
                nc.vector.tensor_scalar_add(dst, acc[:], b_sb[n][:])

            def vtr(b, g):
                """VT [128 e', s] -> V'_h [128 s, (stile, 65)] for group g."""
                for r in range(4):
                    tp = misc_ps.tile([128, 128], BF16, tag="tr",
                                      name=f"vtr_{b}_{g}_{r}")
                    nc.tensor.transpose(
                        tp[:], vt_sbs[b][g][:, r * 128:(r + 1) * 128],
                        ident[:],
                    )
                    for h in range(HEADS):
                        nc.vector.tensor_copy(
                            vp_sbs[b][h][g][:, r * 65:r * 65 + 64],
                            tp[:, h * 64:(h + 1) * 64],
                        )

            pv_tiles = {}   # u -> [h tiles]
            pt_tiles = {}   # (u, st) -> tile

            def sc_quad(u, g):
                """Scores + exp for s-tiles 4g..4g+3 of unit u."""
                b, lq = units[u]
                if g == 0:
                    pv_tiles[u] = [
                        pv_ps.tile([128, 512], F32, tag="pv",
                                   name=f"pv_{u}_{h}") for h in range(HEADS)]
                qt = qt_sbs[b][lq]
                kt = kt_sbs[b][g]
                for r in range(4):
                    st = g * 4 + r
                    sc = sc_ps.tile([128, 2 * LQ], F32, tag="sc",
                                    name=f"sc_{u}_{st}")
                    for h in range(HEADS):
                        nc.tensor.matmul(
                            sc[:, h * LQ:(h + 1) * LQ],
                            lhsT=kt[h * 64:(h + 1) * 64,
                                    r * 128:(r + 1) * 128],
                            rhs=qt[h * 64:(h + 1) * 64, :],
                            start=True, stop=True,
                        )
                    pt = pt_pool.tile([128, 2 * LQ], BF16, tag="pt",
                                      name=f"pt_{u}_{st}")
                    nc.scalar.activation(
                        pt[:], sc[:], mybir.ActivationFunctionType.Exp,
                        scale=0.125,
                    )
                    pt_tiles[(u, st)] = pt

            def pv_wave(u, g, half):
                """Flipped PV matmuls for s-tiles of quad g (half 0/1)."""
                b, lq = units[u]
                pv = pv_tiles[u]
                for r in (0, 1) if half == 0 else (2, 3):
                    st = g * 4 + r
                    pt = pt_tiles[(u, st)]
                    for h in range(HEADS):
                        for c in range(4):
                            # start resets the whole 2KB PSUM bank, so only
                            # the first write of each head-tile may set it.
                            nc.tensor.matmul(
                                pv[h][:, c * 128:c * 128 + 65],
                                lhsT=pt[:, h * LQ + c * 128:
                                        h * LQ + (c + 1) * 128],
                                rhs=vp_sbs[b][h][g][:, r * 65:(r + 1) * 65],
                                start=(st == 0 and c == 0),
                                stop=(st == NST - 1 and c == 3),
                                skip_group_check=True,
                            )

            ot_tiles = {}
            ob_tiles = {}

            def tail_a(u, split=False):
                """Reciprocals, normalize, O transpose for unit u.

                split=True farms half the normalize/copy work to the scalar
                engine — only safe once the exp stream has drained (epilogue).
                """
                pv = pv_tiles[u]
                rec_hc = [[rec_pool.tile([128, 1], F32, tag="rec",
                                         name=f"rec_{u}_{h}_{c}")
                           for c in range(4)] for h in range(HEADS)]
                for h in range(HEADS):
                    for c in range(4):
                        nc.vector.reciprocal(
                            rec_hc[h][c][:],
                            pv[h][:, c * 128 + 64:c * 128 + 65],
                        )
                o_cs = [o_pool.tile([128, 128], BF16, tag="o",
                                    name=f"o_{u}_{c}") for c in range(4)]
                for h in range(HEADS):
                    for c in range(4):
                        if split and h == 1:
                            nc.scalar.mul(
                                o_cs[c][:, h * 64:(h + 1) * 64],
                                pv[h][:, c * 128:c * 128 + 64],
                                rec_hc[h][c][:],
                            )
                        else:
                            nc.vector.tensor_scalar_mul(
                                o_cs[c][:, h * 64:(h + 1) * 64],
                                pv[h][:, c * 128:c * 128 + 64],
                                rec_hc[h][c][:],
                            )
                ot = ot_pool.tile([E2, LQ], BF16, tag="ot", name=f"ot_{u}")
                ot_tiles[u] = ot
                for c in range(4):
                    tr = misc_ps.tile([128, 128], BF16, tag="tr",
                                      name=f"otr_{u}_{c}")
                    nc.tensor.transpose(tr[:], o_cs[c][:], ident[:])
                    if split and c % 2 == 1:
                        nc.scalar.copy(ot[:, c * 128:(c + 1) * 128], tr[:])
                    else:
                        nc.vector.tensor_copy(ot[:, c * 128:(c + 1) * 128],
                                              tr[:])

            def og(u, dt, epi=False):
                """Out-projection group dt of unit u + drain to ob.

                epi=True uses a 3-deep psum ring (borrowing freed sc banks)
                and splits the drain copies between DVE and the scalar
                engine — only safe once the exp stream has drained.
                """
                if dt == 0:
                    ob_tiles[u] = ob_pool.tile([128, NDT, 512], BF16,
                                               tag="ob", name=f"ob_{u}")
                if epi and dt % 3 != 0:
                    big = sc_ps.tile([128, 2 * LQ], F32, tag="sc",
                                     name=f"oge_{u}_{dt}")
                    op = big[:, 0:512]
                else:
                    opt = misc_ps.tile([128, 512], F32, tag="op",
                                       name=f"og_{u}_{dt}")
                    op = opt[:]
                nc.tensor.matmul(
                    op,
                    lhsT=wo_sb[:, dt * 128:(dt + 1) * 128],
                    rhs=ot_tiles[u][:],
                    start=True, stop=True,
                )
                if epi and dt % 2 == 1:
                    nc.scalar.copy(ob_tiles[u][:, dt, :], op)
                else:
                    nc.vector.tensor_copy(ob_tiles[u][:, dt, :], op)

            def out_dma(u):
                b, lq = units[u]
                t0 = b * L + lq * LQ
                nc.sync.dma_start(out=out_t.ap()[:, :, t0:t0 + LQ],
                                  in_=ob_tiles[u][:])

            # ---- fill schedule: thunks run between sc quads ----
            def Fv(b, tt):
                return lambda: proj_tt("v", b, tt)

            def Fqk(n, b, tt):
                return lambda: proj_tt(n, b, tt)

            def Ftr(b, g):
                return lambda: vtr(b, g)

            def Fpv(u, g, half):
                return lambda: pv_wave(u, g, half)

            def Fta(u):
                return lambda: tail_a(u)

            def Fog(u, dt):
                return lambda: og(u, dt)

            def Fdma(u):
                return lambda: out_dma(u)

            F = {(u, g): [] for u in range(8) for g in range(4)}
            # unit 0/1: k-projections and V pipeline paced by DMA arrivals
            F[0, 0] = [Fqk("k", 0, 1)]
            F[0, 1] = [Fqk("k", 0, 2), Fqk("k", 0, 3)]
            F[0, 2] = [Fqk("q", 0, 1)]
            F[0, 3] = [Fv(0, 0), Ftr(0, 0)]
            F[1, 0] = [Fpv(0, 0, 0), Fv(0, 1), Fpv(0, 0, 1), Ftr(0, 1)]
            F[1, 1] = [Fpv(0, 1, 0), Fv(0, 2), Fpv(0, 1, 1), Ftr(0, 2)]
            F[1, 2] = [Fpv(0, 2, 0), Fv(0, 3), Fpv(0, 2, 1), Ftr(0, 3)]
            F[1, 3] = [Fpv(0, 3, 0), Fpv(0, 3, 1), Fta(0), Fqk("q", 0, 2)]
            F[2, 0] = [Fpv(1, 0, 0), Fog(0, 0), Fpv(1, 0, 1), Fog(0, 1)]
            F[2, 1] = [Fpv(1, 1, 0), Fog(0, 2), Fpv(1, 1, 1), Fog(0, 3),
                       Fqk("q", 0, 3)]
            F[2, 2] = [Fpv(1, 2, 0), Fog(0, 4), Fpv(1, 2, 1), Fog(0, 5)]
            F[2, 3] = [Fpv(1, 3, 0), Fog(0, 6), Fpv(1, 3, 1), Fog(0, 7),
                       Fdma(0), Fta(1)]
            F[3, 0] = [Fpv(2, 0, 0), Fog(1, 0), Fpv(2, 0, 1), Fog(1, 1),
                       Fqk("k", 1, 0)]
            F[3, 1] = [Fpv(2, 1, 0), Fog(1, 2), Fpv(2, 1, 1), Fog(1, 3),
                       Fqk("k", 1, 1)]
            F[3, 2] = [Fpv(2, 2, 0), Fog(1, 4), Fpv(2, 2, 1), Fog(1, 5),
                       Fqk("k", 1, 2), Fqk("q", 1, 0)]
            F[3, 3] = [Fpv(2, 3, 0), Fog(1, 6), Fpv(2, 3, 1), Fog(1, 7),
                       Fdma(1), Fta(2), Fqk("k", 1, 3)]
            F[4, 0] = [Fpv(3, 0, 0), Fog(2, 0), Fpv(3, 0, 1), Fog(2, 1)]
            F[4, 1] = [Fpv(3, 1, 0), Fog(2, 2), Fpv(3, 1, 1), Fog(2, 3),
                       Fv(1, 0), Ftr(1, 0), Fqk("q", 1, 1)]
            F[4, 2] = [Fpv(3, 2, 0), Fog(2, 4), Fpv(3, 2, 1), Fog(2, 5),
                       Fv(1, 1), Ftr(1, 1)]
            F[4, 3] = [Fpv(3, 3, 0), Fog(2, 6), Fpv(3, 3, 1), Fog(2, 7),
                       Fdma(2), Fta(3), Fv(1, 2), Ftr(1, 2)]
            F[5, 0] = [Fpv(4, 0, 0), Fog(3, 0), Fpv(4, 0, 1), Fog(3, 1),
                       Fv(1, 3), Ftr(1, 3)]
            F[5, 1] = [Fpv(4, 1, 0), Fog(3, 2), Fpv(4, 1, 1), Fog(3, 3),
                       Fqk("q", 1, 2)]
            F[5, 2] = [Fpv(4, 2, 0), Fog(3, 4), Fpv(4, 2, 1), Fog(3, 5)]
            F[5, 3] = [Fpv(4, 3, 0), Fog(3, 6), Fpv(4, 3, 1), Fog(3, 7),
                       Fdma(3), Fta(4)]
            F[6, 0] = [Fpv(5, 0, 0), Fog(4, 0), Fpv(5, 0, 1), Fog(4, 1),
                       Fqk("q", 1, 3)]
            F[6, 1] = [Fpv(5, 1, 0), Fog(4, 2), Fpv(5, 1, 1), Fog(4, 3)]
            F[6, 2] = [Fpv(5, 2, 0), Fog(4, 4), Fpv(5, 2, 1), Fog(4, 5)]
            F[6, 3] = [Fpv(5, 3, 0), Fog(4, 6), Fpv(5, 3, 1), Fog(4, 7),
                       Fdma(4), Fta(5)]
            F[7, 0] = [Fpv(6, 0, 0), Fog(5, 0), Fpv(6, 0, 1), Fog(5, 1)]
            F[7, 1] = [Fpv(6, 1, 0), Fog(5, 2), Fpv(6, 1, 1), Fog(5, 3)]
            F[7, 2] = [Fpv(6, 2, 0), Fog(5, 4), Fpv(6, 2, 1), Fog(5, 5)]
            F[7, 3] = [Fpv(6, 3, 0), Fog(5, 6), Fpv(6, 3, 1), Fog(5, 7),
                       Fdma(5), Fta(6)]

            # ---- PE warmup: keep the ramp streak alive until K0 lands ----
            dummy = const.tile([128, 128], BF16, tag="dummy")
            nc.vector.memset(dummy[:], 0.0)
            warm_ps = misc_ps.tile([128, 512], F32, tag="op", name="warm_ps")
            for i in range(28):
                nc.tensor.matmul(warm_ps[:, 0:128], lhsT=dummy[:],
                                 rhs=dummy[:], start=True, stop=True)

            # ---- prologue projections ----
            proj_tt("k", 0, 0)
            proj_tt("q", 0, 0)

            # ---- main pipelined emission ----
            for u in range(8):
                for g in range(4):
                    sc_quad(u, g)
                    for thunk in F[(u, g)]:
                        thunk()

            # ---- epilogue ----
            pv_wave(7, 0, 0)
            og(6, 0)
            pv_wave(7, 0, 1)
            og(6, 1)
            pv_wave(7, 1, 0)
            og(6, 2)
            pv_wave(7, 1, 1)
            og(6, 3)
            pv_wave(7, 2, 0)
            og(6, 4)
            pv_wave(7, 2, 1)
            og(6, 5)
            pv_wave(7, 3, 0)
            og(6, 6)
            pv_wave(7, 3, 1)
            og(6, 7)
            out_dma(6)
            tail_a(7, split=True)
            for dt in range(NDT):
                og(7, dt, epi=True)
            out_dma(7)

    nc.compile()
    return nc


def _get_nc():
    global _CACHED_NC
    if _CACHED_NC is None:
        _CACHED_NC = build_nc()
    return _CACHED_NC


def _prep_inputs(queries, keys, values, Wq, bq, Wk, bk, Wv, bv, Wo, bo):
    bf16 = ml_dtypes.bfloat16
    x_t = {}
    for n, arr in (("q", queries), ("k", keys), ("v", values)):
        # X^T [D, TOK] -> [128, NKT, TOK]: row p of kt-block kt is X^T row
        # kt*128+p
        full = np.asarray(arr, np.float32).reshape(TOK, D).T
        x_t[n] = np.ascontiguousarray(
            full.reshape(NKT, 128, TOK).transpose(1, 0, 2)
        ).astype(bf16)
    in_maps = []
    for c in range(NCORES):
        sl = slice(c * E2, (c + 1) * E2)
        m = {
            "xq_t": x_t["q"], "xk_t": x_t["k"], "xv_t": x_t["v"],
            "wq": _warrange(np.asarray(Wq, np.float32)[:, sl]),
            "wk": _warrange(np.asarray(Wk, np.float32)[:, sl]),
            "wv": _warrange(np.asarray(Wv, np.float32)[:, sl]),
            "bq": np.ascontiguousarray(np.asarray(bq, np.float32)[sl].reshape(E2, 1)),
            "bk": np.ascontiguousarray(np.asarray(bk, np.float32)[sl].reshape(E2, 1)),
            "bv": np.ascontiguousarray(np.asarray(bv, np.float32)[sl].reshape(E2, 1)),
            "wo": np.ascontiguousarray(np.asarray(Wo, np.float32)[sl, :]).astype(bf16),
            "ident_in": _IDENT,
        }
        in_maps.append(m)
    return in_maps


def _postprocess(results, bo):
    acc = np.zeros((128, NDT, TOK), np.float64)
    for r in results:
        acc += r["out_t"].astype(np.float64)  # bf16 partials, summed in fp64
    # [128 p, 8 dt, TOK] -> [D, TOK] with d = dt*128 + p
    full = acc.transpose(1, 0, 2).reshape(D, TOK)
    out = full.T.astype(np.float32) + np.asarray(bo, np.float32)[None, :]
    return out.reshape(B, L, D)


def run(trace=False, **inputs):
    nc = _get_nc()
    in_maps = _prep_inputs(**inputs)
    res = run_bass_kernel_spmd(nc, in_maps, core_ids=list(range(NCORES)),
                               trace=trace)
    out = _postprocess(res.results, inputs["bo"])
    return out, res


def kernel(**inputs):
    out, _ = run(trace=False, **inputs)
    return out


# revision 56
# speedup vs baseline: 1.2542x; 1.2542x over previous
"""Multi-head attention (B=2, L=S=2048, D=1024, H=16, E=64) on 8 TRN2 cores.

Sharding: 8 cores = 2 batches x 4 head-groups. Core c owns batch c//4 and
head pair-group c%4 (heads 4(c%4)..4(c%4)+3, processed as two 128-wide
pairs). Each core reads only its batch's (host-pre-transposed) q/k/v —
14MB instead of 25MB — and the second pair's projections reuse the X
tiles already resident in SBUF. Each core writes bf16 partials of the
output projection per (pair, token); the host sums pairs and the four
cores per batch, then adds bo (with bv @ Wo folded in, since attention
rows sum to 1).

Software-pipelined emission built around the scalar-engine exp being the
pacing engine (~133us of exp vs ~139us of matmul):
  X^T  host-supplied as [128, 8 kt, 2048 tok] bf16 per batch; DMA'd per
       512-token chunk (256-token half-chunks on latency-critical paths).
  QT/KT = (XW_p)^T      [128 e', 512] bf16 SBUF (4 tiles per pair)
  V'_h = [V_h | 1]      [128 s, 4*65] bf16, projected directly in
       [token, e] orientation (lhsT = X^T chunk); no transpose, no bias.
  scores^T (per s-tile) [128 s, 2*512 (h,l)] f32 PSUM
  P^T = exp(s/8)        [128 s, 2*512] bf16 SBUF (ScalarE)
  PV flipped: lhsT=P^T chunk [128 s, 128 l], rhs=V'_h [128 s, 65]
      -> pv[h] [128 l, 4x(65 pad 128)] PSUM accumulated over 16 s-tiles;
      col 64 is the softmax denominator (ones column of V'). Free-dim 65
      instead of 512 halves PE time for PV. PSUM start flags reset a
      whole 2KB bank, so only the first write of each bank sets start.
  normalize: per-partition reciprocal + scalar mult (DVE)
      -> O [128 l, 128 e'] bf16; PE-transpose -> OT [128 e', 512] bf16.
  out-proj: Wo_p.T @ OT -> [128 d, 512] PSUM -> DVE copy into ob
      [128, 8, 512] bf16 -> batched 3D DMA per unit (SP queue).
PSUM: scores 4 banks + pv 2 + shared outproj/transpose ring 2 (transpose
targets are bitcast views of the f32 ring tiles). DMA issue never touches
the scalar queue, so exp dispatch cannot stall behind descriptor
generation. Units are emitted as 4 quads of (8 score matmuls + 4 exps),
with all other work (PV waves lagging one unit, projections, previous
units' tails) interleaved between quads against measured DMA arrival
times; the epilogue splits drain copies across DVE and the then-idle
scalar engine on two independent PSUM rings.
"""

import numpy as np
import ml_dtypes

import concourse.bass as bass
import concourse.bacc as bacc
import concourse.mybir as mybir
from concourse.tile import TileContext
from concourse.bass_utils import run_bass_kernel_spmd

BF16 = mybir.dt.bfloat16
F32 = mybir.dt.float32
F32R = mybir.dt.float32r

B, L, D = 2, 2048, 1024
TOK = B * L              # 4096 (output token axis: pair*2048+t)
TOKC = L                 # per-core input tokens (one batch)
H, E = 16, 64
NCORES = 8
E2 = 128                 # projection output dims per core (2 heads)
NKT = D // 128           # 8 k-tiles of the contraction
LQ = 512                 # l-quarter: query-token tile inside attention
NLQ = L // LQ            # 4 per batch
NST = L // 128           # 16 s-tiles (key tokens) per batch
HEADS = 2                # heads per core
NDT = D // 128           # 8 output-row tiles

_CACHED_NC = None
_IDENT = np.eye(128, dtype=ml_dtypes.bfloat16)


def _warrange(w):
    # [D, E2] -> [128, NKT*E2]: row p holds [w[kt*128+p, :] for kt]
    return np.ascontiguousarray(
        w.reshape(NKT, 128, E2).transpose(1, 0, 2).reshape(128, NKT * E2)
    ).astype(ml_dtypes.bfloat16)


def build_nc():
    nc = bacc.Bacc("TRN2", target_bir_lowering=False)

    xt = {n: nc.declare_dram_parameter(f"x{n}_t", [128, NKT, TOKC], BF16,
                                       isOutput=False)
          for n in ("q", "k", "v")}
    w = {n: nc.declare_dram_parameter(f"w{n}", [128, 2 * NKT * E2], BF16,
                                      isOutput=False)
         for n in ("q", "k", "v")}
    bias = {n: nc.declare_dram_parameter(f"b{n}", [E2, 2], F32, isOutput=False)
            for n in ("q", "k")}
    wo = nc.declare_dram_parameter("wo", [E2, 2 * D], BF16, isOutput=False)
    ident_in = nc.declare_dram_parameter("ident_in", [128, 128], BF16,
                                         isOutput=False)
    out_t = nc.declare_dram_parameter("out_t", [128, NDT, TOK], BF16,
                                      isOutput=True)

    units = [(0, lq) for lq in range(NLQ)] + [(1, lq) for lq in range(NLQ)]

    with TileContext(nc) as tc:
        with (
            tc.tile_pool(name="const", bufs=1) as const,
            tc.tile_pool(name="persist", bufs=1) as persist,
            tc.tile_pool(name="xt_pool", bufs=9) as xt_pool,
            tc.tile_pool(name="xth_pool", bufs=20) as xth_pool,
            tc.tile_pool(name="pt_pool", bufs=18) as pt_pool,
            tc.tile_pool(name="o_pool", bufs=8) as o_pool,
            tc.tile_pool(name="rec_pool", bufs=16) as rec_pool,
            tc.tile_pool(name="ot_pool", bufs=3) as ot_pool,
            tc.tile_pool(name="ob_pool", bufs=2) as ob_pool,
            tc.tile_pool(name="sc_ps", bufs=2, space="PSUM") as sc_ps,
            tc.tile_pool(name="pv_ps", bufs=2, space="PSUM") as pv_ps,
            tc.tile_pool(name="misc_ps", bufs=2, space="PSUM") as misc_ps,
        ):
            # ---- constants ----
            ident = const.tile([128, 128], BF16, tag="ident")
            w_sb = {n: const.tile([128, 2 * NKT * E2], BF16, tag=f"w_{n}",
                                  name=f"w_{n}") for n in ("q", "k", "v")}
            b_sb = {}
            for n in ("q", "k"):
                b_dma = const.tile([E2, 2], F32, tag=f"bdma_{n}",
                                   name=f"bdma_{n}")
                nc.gpsimd.dma_start(out=b_dma[:], in_=bias[n].ap())
                b_sb[n] = const.tile([E2, 2], F32, tag=f"b_{n}", name=f"b_{n}")
                nc.vector.tensor_copy(b_sb[n][:], b_dma[:])

            warm = const.tile([1, 2], F32, tag="warm")
            nc.vector.memset(warm[:], 0.0)
            nc.scalar.activation(warm[:], warm[:],
                                 mybir.ActivationFunctionType.Exp)
            wo_sb = const.tile([E2, 2 * D], BF16, tag="wo")

            qt_sbs = [[persist.tile([E2, LQ], BF16, tag=f"qt{b}_{t}",
                                    name=f"qt{b}_{t}") for t in range(4)]
                      for b in range(B)]
            kt_sbs = [[persist.tile([E2, LQ], BF16, tag=f"kt{b}_{t}",
                                    name=f"kt{b}_{t}") for t in range(4)]
                      for b in range(B)]
            # V' per head: [128 spart, (4 stile, 65)] with col 64 == 1.0
            vp_sbs = [[[persist.tile([128, 4 * 65], BF16, tag=f"vp{b}_{h}_{g}",
                                     name=f"vp{b}_{h}_{g}") for g in range(4)]
                       for h in range(HEADS)] for b in range(B)]

            for h in range(HEADS):
                nc.vector.memset(vp_sbs[0][h][0][:], 1.0)

            nc.gpsimd.dma_start(out=wo_sb[:], in_=wo.ap())
            nc.gpsimd.dma_start(out=ident[:], in_=ident_in.ap())

            proj_out = {"q": qt_sbs, "k": kt_sbs}

            # ---- DMA issue (SP queue), in arrival-need order ----
            x_sb = {}

            def dma_w(n, p=0):
                mark(f"dmaw_{n}{p}")
                s = p * NKT * E2
                nc.sync.dma_start(out=w_sb[n][:, s:s + NKT * E2],
                                  in_=w[n].ap()[:, s:s + NKT * E2])

            def dma_x(n, b, tt):
                mark(f"dmax_{n}{b}_{tt}")
                t0 = tt * 512
                xtile = xt_pool.tile([128, NKT, 512], BF16, tag="xt",
                                     name=f"x_{n}{b}_{tt}")
                nc.sync.dma_start(out=xtile[:],
                                  in_=xt[n].ap()[:, :, t0:t0 + 512])
                x_sb[(n, tt)] = xtile

            def dma_x_half(n, b, tt, half):
                mark(f"dmaxh_{n}{b}_{tt}_{half}")
                t0 = tt * 512 + half * 256
                xtile = xth_pool.tile([128, NKT, 256], BF16, tag="xth",
                                     name=f"xh_{n}{b}_{tt}_{half}")
                nc.sync.dma_start(out=xtile[:],
                                  in_=xt[n].ap()[:, :, t0:t0 + 256])
                x_sb[(n, tt, half)] = xtile

            def proj_tt_halves(n, b, tt):
                """Like proj_tt but from two half tiles; single psum bank, so
                only the very first matmul may set start (bank reset)."""
                acc = misc_ps.tile([128, 512], F32, tag="op",
                                   name=f"projh_{n}{b}_{tt}")
                dst = proj_out[n][b][tt]
                for half in range(2):
                    c0 = half * 256
                    xtile = x_sb[(n, tt, half)]
                    for kt in range(NKT):
                        nc.tensor.matmul(
                            acc[:, c0:c0 + 256],
                            lhsT=w_sb[n][:, b * NKT * E2 + kt * E2:
                                         b * NKT * E2 + (kt + 1) * E2],
                            rhs=xtile[:, kt, :],
                            start=(half == 0 and kt == 0),
                            stop=(half == 1 and kt == NKT - 1),
                            skip_group_check=True,
                        )
                    nc.vector.tensor_scalar_add(dst[:, c0:c0 + 256],
                                                acc[:, c0:c0 + 256],
                                                b_sb[n][:, b:b + 1])

            dma_w("k")
            dma_x_half("k", 0, 0, 0)
            dma_w("q")
            dma_x_half("q", 0, 0, 0)
            dma_x_half("q", 0, 0, 1)
            dma_x_half("k", 0, 0, 1)
            dma_x_half("k", 0, 1, 0)
            dma_x_half("k", 0, 1, 1)
            dma_w("v")
            dma_x_half("k", 0, 2, 0)
            dma_x_half("k", 0, 2, 1)
            dma_x_half("k", 0, 3, 0)
            dma_x_half("k", 0, 3, 1)
            dma_x_half("q", 0, 1, 0)
            dma_x_half("q", 0, 1, 1)
            dma_x_half("v", 0, 0, 0)
            dma_x_half("v", 0, 0, 1)
            dma_x_half("v", 0, 1, 0)
            dma_x_half("v", 0, 1, 1)
            dma_x_half("v", 0, 2, 0)
            dma_x_half("v", 0, 2, 1)
            dma_x("q", 0, 2)
            dma_x_half("v", 0, 3, 0)
            dma_x_half("v", 0, 3, 1)
            dma_x("q", 0, 3)
            dma_w("k", 1)
            dma_w("q", 1)
            dma_w("v", 1)

            # ---- emission helpers ----
            def proj_tt(n, b, tt):
                """Project one 512-token chunk: 8 accumulating matmuls."""
                acc = misc_ps.tile([128, 512], F32, tag="op",
                                   name=f"proj_{n}{b}_{tt}")
                xtile = x_sb[(n, tt)]
                for kt in range(NKT):
                    nc.tensor.matmul(
                        acc[:],
                        lhsT=w_sb[n][:, b * NKT * E2 + kt * E2:
                                     b * NKT * E2 + (kt + 1) * E2],
                        rhs=xtile[:, kt, :],
                        start=(kt == 0),
                        stop=(kt == NKT - 1),
                    )
                if n == "v":
                    dst = vt_sbs[b][tt][:]
                else:
                    dst = proj_out[n][b][tt][:]
                nc.vector.tensor_scalar_add(dst, acc[:], b_sb[n][:])

            def vtr(b, g):
                """VT [128 e', s] -> V'_h [128 s, (stile, 65)] for group g."""
                for r in range(4):
                    tp = misc_ps.tile([128, 128], BF16, tag="tr",
                                      name=f"vtr_{b}_{g}_{r}")
                    nc.tensor.transpose(
                        tp[:], vt_sbs[b][g][:, r * 128:(r + 1) * 128],
                        ident[:],
                    )
                    for h in range(HEADS):
                        nc.vector.tensor_copy(
                            vp_sbs[b][h][g][:, r * 65:r * 65 + 64],
                            tp[:, h * 64:(h + 1) * 64],
                        )

            pv_tiles = {}   # u -> [h tiles]
            pt_tiles = {}   # (u, st) -> tile

            def sc_quad(u, g):
                """Scores + exp for s-tiles 4g..4g+3 of unit u."""
                b, lq = units[u]
                if g == 0:
                    pv_tiles[u] = [
                        pv_ps.tile([128, 512], F32, tag="pv",
                                   name=f"pv_{u}_{h}") for h in range(HEADS)]
                qt = qt_sbs[b][lq]
                kt = kt_sbs[b][g]
                for r in range(4):
                    st = g * 4 + r
                    sc = sc_ps.tile([128, 2 * LQ], F32, tag="sc",
                                    name=f"sc_{u}_{st}")
                    for h in range(HEADS):
                        nc.tensor.matmul(
                            sc[:, h * LQ:(h + 1) * LQ],
                            lhsT=kt[h * 64:(h + 1) * 64,
                                    r * 128:(r + 1) * 128],
                            rhs=qt[h * 64:(h + 1) * 64, :],
                            start=True, stop=True,
                        )
                    pt = pt_pool.tile([128, 2 * LQ], BF16, tag="pt",
                                      name=f"pt_{u}_{st}")
                    nc.scalar.activation(
                        pt[:], sc[:], mybir.ActivationFunctionType.Exp,
                        scale=0.125,
                    )
                    pt_tiles[(u, st)] = pt

            def pv_wave(u, g, half):
                """Flipped PV matmuls for s-tiles of quad g (half 0/1)."""
                b, lq = units[u]
                pv = pv_tiles[u]
                for r in (0, 1) if half == 0 else (2, 3):
                    st = g * 4 + r
                    pt = pt_tiles[(u, st)]
                    for h in range(HEADS):
                        for c in range(4):
                            # start resets the whole 2KB PSUM bank, so only
                            # the first write of each head-tile may set it.
                            nc.tensor.matmul(
                                pv[h][:, c * 128:c * 128 + 65],
                                lhsT=pt[:, h * LQ + c * 128:
                                        h * LQ + (c + 1) * 128],
                                rhs=vp_sbs[b][h][g][:, r * 65:(r + 1) * 65],
                                start=(st == 0 and c == 0),
                                stop=(st == NST - 1 and c == 3),
                                skip_group_check=True,
                            )

            ot_tiles = {}
            ob_tiles = {}

            def tail_a(u, split=False):
                """Reciprocals, normalize, O transpose for unit u.

                split=True farms half the normalize/copy work to the scalar
                engine — only safe once the exp stream has drained (epilogue).
                """
                pv = pv_tiles[u]
                if split:
                    # scalar engine needs reciprocals; only for its half
                    rec_h1 = [rec_pool.tile([128, 1], F32, tag="rec",
                                            name=f"rec_{u}_{c}")
                              for c in range(4)]
                    for c in range(4):
                        nc.vector.reciprocal(
                            rec_h1[c][:],
                            pv[1][:, c * 128 + 64:c * 128 + 65])
                o_cs = [o_pool.tile([128, 128], BF16, tag="o",
                                    name=f"o_{u}_{c}") for c in range(4)]
                for h in range(HEADS):
                    for c in range(4):
                        if split and h == 1:
                            nc.scalar.mul(
                                o_cs[c][:, h * 64:(h + 1) * 64],
                                pv[h][:, c * 128:c * 128 + 64],
                                rec_h1[c][:],
                            )
                        else:
                            nc.vector.tensor_scalar(
                                o_cs[c][:, h * 64:(h + 1) * 64],
                                pv[h][:, c * 128:c * 128 + 64],
                                pv[h][:, c * 128 + 64:c * 128 + 65],
                                None,
                                op0=mybir.AluOpType.divide,
                            )
                o_tiles[u] = o_cs

            def tail_t(u, split=False):
                """O transposes -> OT for unit u (PE + copies)."""
                mark(f"tailT_{u}")
                o_cs = o_tiles[u]
                ot = ot_pool.tile([E2, LQ], BF16, tag="ot", name=f"ot_{u}")
                ot_tiles[u] = ot
                for c in range(4):
                    tr = tr_view(f"otr_{u}_{c}")
                    nc.tensor.transpose(tr, o_cs[c][:], ident[:])
                    if split and c % 2 == 1:
                        nc.scalar.copy(ot[:, c * 128:(c + 1) * 128], tr)
                    else:
                        nc.vector.tensor_copy(ot[:, c * 128:(c + 1) * 128],
                                              tr)

            def og(u, dt, epi=False):
                """Out-projection group dt of unit u + drain to ob.

                epi=True uses a 3-deep psum ring (borrowing freed sc banks)
                and splits the drain copies between DVE and the scalar
                engine — only safe once the exp stream has drained.
                """
                if dt == 0:
                    ob_tiles[u] = ob_pool.tile([128, NDT, 512], BF16,
                                               tag="ob", name=f"ob_{u}")
                if epi == "act" or (epi is True and dt % 3 != 0):
                    big = sc_ps.tile([128, 2 * LQ], F32, tag="sc",
                                     name=f"oge_{u}_{dt}")
                    op = big[:, 0:512]
                else:
                    opt = misc_ps.tile([128, 512], F32, tag="op",
                                       name=f"og_{u}_{dt}")
                    op = opt[:]
                p = units[u][0]
                nc.tensor.matmul(
                    op,
                    lhsT=wo_sb[:, p * D + dt * 128:p * D + (dt + 1) * 128],
                    rhs=ot_tiles[u][:],
                    start=True, stop=True,
                )
                if epi == "act":
                    nc.scalar.copy(ob_tiles[u][:, dt, :], op)
                elif epi and dt % 2 == 1:
                    nc.scalar.copy(ob_tiles[u][:, dt, :], op)
                else:
                    nc.vector.tensor_copy(ob_tiles[u][:, dt, :], op)

            def out_dma(u):
                b, lq = units[u]
                t0 = b * L + lq * LQ
                nc.sync.dma_start(out=out_t.ap()[:, :, t0:t0 + LQ],
                                  in_=ob_tiles[u][:])

            # ---- fill schedule: thunks run between sc quads ----
            def Fv(b, tt):
                return lambda: vproj_direct(b, tt)

            def vproj_direct_half(b, tt, half):
                """Half-chunk direct V projection from an xth tile."""
                mark(f"vdh_{b}_{tt}_{half}")
                acc = misc_ps.tile([128, 512], F32, tag="op",
                                   name=f"vdh_{b}_{tt}_{half}")
                xtile = x_sb[("v", tt, half)]
                for ci in range(2):
                    c = half * 2 + ci
                    for kt in range(NKT):
                        nc.tensor.matmul(
                            acc[:, ci * 128:(ci + 1) * 128],
                            lhsT=xtile[:, kt, ci * 128:(ci + 1) * 128],
                            rhs=w_sb["v"][:, b * NKT * E2 + kt * E2:
                                          b * NKT * E2 + (kt + 1) * E2],
                            start=(ci == 0 and kt == 0),
                            stop=(ci == 1 and kt == NKT - 1),
                            skip_group_check=True,
                        )
                for ci in range(2):
                    c = half * 2 + ci
                    for h in range(HEADS):
                        nc.vector.tensor_copy(
                            vp_sbs[b][h][tt][:, c * 65:c * 65 + 64],
                            acc[:, ci * 128 + h * 64:ci * 128 + h * 64 + 64],
                        )

            def Fvh(b, tt, half):
                return lambda: vproj_direct_half(b, tt, half)

            def Fqk(n, b, tt):
                return lambda: proj_tt(n, b, tt)

            def Fqkh(n, b, tt):
                return lambda: proj_tt_halves(n, b, tt)

            def Fpv(u, g, half):
                return lambda: pv_wave(u, g, half)

            def Ftn(u):
                return lambda: tail_n(u)

            def Ftt(u):
                return lambda: tail_t(u)

            def Fmemset(b, gs):
                def f():
                    for h in range(HEADS):
                        for g in gs:
                            nc.vector.memset(vp_sbs[b][h][g][:], 1.0)
                return f

            def Fog(u, dt):
                return lambda: og(u, dt)

            def Fdma(u, half=None):
                return lambda: out_dma(u, half=half)

            F = {(u, g): [] for u in range(8) for g in range(4)}
            # unit 0/1: k-projections and V pipeline paced by DMA arrivals
            F[0, 0] = [Fmemset(0, (1, 2, 3)), Fqkh("k", 0, 1), Fqkh("k", 0, 2)]
            F[0, 1] = [Fqkh("k", 0, 3)]
            F[0, 2] = [Fqkh("q", 0, 1), Fmemset(1, (0, 1, 2, 3))]
            F[0, 3] = [Fvh(0, 0, 0), Fvh(0, 0, 1)]
            F[1, 0] = [Fpv(0, 0, 0), Fvh(0, 1, 0), Fpv(0, 0, 1), Fvh(0, 1, 1)]
            F[1, 1] = [Fpv(0, 1, 0), Fvh(0, 2, 0), Fpv(0, 1, 1),
                       Fvh(0, 2, 1), Fqk("q", 0, 2)]
            F[1, 2] = [Fpv(0, 2, 0), Fvh(0, 3, 0), Fpv(0, 2, 1), Fvh(0, 3, 1)]
            F[1, 3] = [Fpv(0, 3, 0), Fpv(0, 3, 1), Ftn(0), Fqk("q", 0, 3)]
            F[2, 0] = [Ftt(0), Fpv(1, 0, 0), Fpv(1, 0, 1)]
            F[2, 1] = [Fpv(1, 1, 0), Fog(0, 0), Fpv(1, 1, 1), Fog(0, 1)]
            F[2, 2] = [Fpv(1, 2, 0), Fog(0, 2), Fpv(1, 2, 1), Fqk("k", 1, 0)]
            F[2, 3] = [Fpv(1, 3, 0), Fog(0, 3), Fpv(1, 3, 1), Fog(0, 4),
                       Ftn(1), Fqk("k", 1, 1)]
            F[3, 0] = [Ftt(1), Fpv(2, 0, 0), Fog(0, 5), Fpv(2, 0, 1),
                       Fog(0, 6), Fog(0, 7), Fdma(0)]
            F[3, 1] = [Fpv(2, 1, 0), Fog(1, 0), Fpv(2, 1, 1), Fqk("k", 1, 2)]
            F[3, 2] = [Fpv(2, 2, 0), Fog(1, 1), Fpv(2, 2, 1), Fqk("q", 1, 0)]
            F[3, 3] = [Fpv(2, 3, 0), Fog(1, 2), Fpv(2, 3, 1), Fog(1, 3),
                       Ftn(2), Fqk("k", 1, 3)]
            F[4, 0] = [Ftt(2), Fpv(3, 0, 0), Fog(1, 4), Fpv(3, 0, 1),
                       Fog(1, 5)]
            F[4, 1] = [Fpv(3, 1, 0), Fog(1, 6), Fpv(3, 1, 1), Fog(1, 7),
                       Fdma(1), Fv(1, 0)]
            F[4, 2] = [Fpv(3, 2, 0), Fog(2, 0), Fpv(3, 2, 1), Fog(2, 1),
                       Fqk("q", 1, 1)]
            F[4, 3] = [Fpv(3, 3, 0), Fog(2, 2), Fpv(3, 3, 1),
                       Ftn(3), Fv(1, 1)]
            F[5, 0] = [Ftt(3), Fpv(4, 0, 0), Fpv(4, 0, 1), Fv(1, 2)]
            F[5, 1] = [Fpv(4, 1, 0), Fpv(4, 1, 1),
                       Fqk("q", 1, 2)]
            F[5, 2] = [Fpv(4, 2, 0), Fpv(4, 2, 1),
                       Fv(1, 3)]
            F[5, 3] = [Fpv(4, 3, 0), Fog(2, 3), Fpv(4, 3, 1),
                       Fog(2, 4), Fog(2, 5), Fdma(2, 0), Ftn(4)]
            F[6, 0] = [Ftt(4), Fpv(5, 0, 0), Fog(2, 6), Fpv(5, 0, 1),
                       Fog(2, 7), Fdma(2, 1)]
            F[6, 1] = [Fpv(5, 1, 0), Fog(3, 0), Fpv(5, 1, 1), Fog(3, 1),
                       Fog(3, 2)]
            F[6, 2] = [Fpv(5, 2, 0), Fog(3, 3), Fpv(5, 2, 1), Fog(3, 4),
                       Fog(3, 5), Fqk("q", 1, 3)]
            F[6, 3] = [Fpv(5, 3, 0), Fog(3, 6), Fog(3, 7), Fdma(3),
                       Fpv(5, 3, 1), Fog(4, 0), Ftn(5)]
            F[7, 0] = [Ftt(5), Fpv(6, 0, 0), Fog(4, 1), Fpv(6, 0, 1),
                       Fog(4, 2)]
            F[7, 1] = [Fpv(6, 1, 0), Fog(4, 3), Fpv(6, 1, 1), Fog(4, 4)]
            F[7, 2] = [Fpv(6, 2, 0), Fog(4, 5), Fpv(6, 2, 1), Fog(4, 6),
                       Fog(4, 7), Fdma(4)]
            F[7, 3] = [Fpv(6, 3, 0), Fog(5, 0), Fpv(6, 3, 1), Fog(5, 1),
                       Fog(5, 2), Ftn(6)]

            # ---- prologue projections ----
            proj_tt_halves("k", 0, 0)
            proj_tt_halves("q", 0, 0)

            # ---- main pipelined emission ----
            for u in range(8):
                for g in range(4):
                    sc_quad(u, g)
                    for thunk in F[(u, g)]:
                        thunk()

            # ---- epilogue ----
            tail_t(6)
            pv_wave(7, 0, 0)
            og(5, 3)
            pv_wave(7, 0, 1)
            og(5, 4)
            pv_wave(7, 1, 0)
            og(5, 5)
            pv_wave(7, 1, 1)
            og(5, 6)
            pv_wave(7, 2, 0)
            og(5, 7)
            pv_wave(7, 2, 1)
            out_dma(5)
            pv_wave(7, 3, 0)
            pv_wave(7, 3, 1)
            tail_n(7, split=True)
            tail_t(7, split=True)
            for dt in range(4):
                og(6, dt)            # misc ring, DVE copies
                og(7, dt, epi="act")  # sc ring, Act copies
            out_dma(6, half=0)
            out_dma(7, half=0)
            for dt in range(4, NDT):
                og(6, dt)
                og(7, dt, epi="act")
            out_dma(6, half=1)
            out_dma(7, half=1)

    nc.compile()
    return nc


def _get_nc():
    global _CACHED_NC
    if _CACHED_NC is None:
        _CACHED_NC = build_nc()
    return _CACHED_NC


def _prep_inputs(queries, keys, values, Wq, bq, Wk, bk, Wv, bv, Wo, bo):
    bf16 = ml_dtypes.bfloat16
    x_t = {}
    for n, arr in (("q", queries), ("k", keys), ("v", values)):
        full = np.asarray(arr, np.float32).reshape(B, L, D)
        x_t[n] = [np.ascontiguousarray(
            full[b].T.reshape(NKT, 128, L).transpose(1, 0, 2)).astype(bf16)
            for b in range(B)]
    in_maps = []
    for c in range(NCORES):
        b, hg = c // 4, c % 4
        m = {"xq_t": x_t["q"][b], "xk_t": x_t["k"][b], "xv_t": x_t["v"][b],
             "ident_in": _IDENT}
        for nm, W in (("wq", Wq), ("wk", Wk), ("wv", Wv)):
            Wf = np.asarray(W, np.float32)
            m[nm] = np.ascontiguousarray(np.concatenate(
                [_warrange(Wf[:, (hg * 2 + p) * E2:(hg * 2 + p + 1) * E2])
                 for p in range(2)], axis=1))
        for nm, bb in (("bq", bq), ("bk", bk)):
            bf = np.asarray(bb, np.float32)
            m[nm] = np.ascontiguousarray(np.stack(
                [bf[(hg * 2 + p) * E2:(hg * 2 + p + 1) * E2]
                 for p in range(2)], axis=1))
        Wof = np.asarray(Wo, np.float32)
        m["wo"] = np.ascontiguousarray(np.concatenate(
            [Wof[(hg * 2 + p) * E2:(hg * 2 + p + 1) * E2, :]
             for p in range(2)], axis=1)).astype(bf16)
        in_maps.append(m)
    return in_maps


def _postprocess(results, bo, bv, Wo):
    # bv folded here: rows of the attention matrix sum to 1, so
    # Attn(V + 1 bv^T) Wo = Attn(V) Wo + 1 (bv Wo); fold bv Wo into bo.
    bo_eff = (np.asarray(bo, np.float64) +
              np.asarray(bv, np.float64) @ np.asarray(Wo, np.float64))
    accs = [np.zeros((128, NDT, TOK), np.float64) for _ in range(B)]
    for c, r in enumerate(results):
        accs[c // 4] += r["out_t"].astype(np.float64)
    out = np.zeros((B, L, D), np.float32)
    for b in range(B):
        full = accs[b].transpose(1, 0, 2).reshape(D, TOK)
        batch = full[:, 0:L] + full[:, L:2 * L]  # sum the two head-pairs
        out[b] = (batch.T + bo_eff[None, :]).astype(np.float32)
    return out


def run(trace=False, **inputs):
    nc = _get_nc()
    in_maps = _prep_inputs(**inputs)
    res = run_bass_kernel_spmd(nc, in_maps, core_ids=list(range(NCORES)),
                               trace=trace)
    out = _postprocess(res.results, inputs["bo"], inputs["bv"], inputs["Wo"])
    return out, res


def kernel(**inputs):
    out, _ = run(trace=False, **inputs)
    return out
